# revision 1
# baseline (speedup 1.0000x reference)
"""MoE expert-collection grouped GEMM for Trainium2, expert-parallel over 8
NeuronCores, fp8 DoubleRow matmuls.

Problem (hardcoded shapes):
  sorted_features  [65536, 1024] f32   tokens sorted by expert, 4096/expert
  expert_ids_sorted[65536] i32         unused: split is static equal-count
  routing_matrix   [1024, 2048, 16] f32
  routing_bias     [2048, 16] f32
  out = silu(x_e @ W_e + b_e) per expert  -> [65536, 2048] f32

Sharding: expert-parallel, 2 experts (= 8192 contiguous sorted tokens) per
core. Host-side dispatch hands each core its token block transposed
(feature-major) and scaled by S_X in fp8 e4m3, its 2 experts' weights scaled
by S_W in fp8 e4m3 (DRAM pre-laid in SBUF tile order: 4KB per-partition
lines), and bias pre-scaled by S_X*S_W broadcast to 128 partitions in fp16.
The matmul term has std ~0.17 vs bias std 1.0, so fp8 noise lands well
under the 2e-2 gate.

Device pipeline per core: DoubleRow fp8 matmuls (two 128-row k-blocks per
instruction, 2 MACs/cell/cycle) accumulating fp32 in PSUM, k-pair-outer /
out-block-inner so one stationary x tile serves 4 matmuls. A tsub's 4 PSUM
banks accumulate while the other 4 drain through one batched DVE add
(+prescaled fp16 bias) and one batched ACT Silu (scale=1/(S_X*S_W))
writing fp16; host upcasts to fp32.

Schedule shaping:
- 6 zero-matmul warmups flip the PE HAM clock-gate to 2.4GHz while the
  critical preload streams, so real matmuls start warm.
- weight/x loads are need-ordered across the two HWDGE rings; expert-0
  bias is deferred behind a dependency on w-kp1 so its descriptors don't
  steal head-window queue slots from the critical weight stream.
- y stores ride the scalar ring (a store on the sync ring blocks the
  in-order sequencer and delays x prefetch issue) except the last two
  token tiles, whose stores move to the then-idle sync ring.
- the final token tile runs ob-outer with quarter drains chained behind
  each bank's last matmul; the one before it uses half drains, so the
  tail is a short pipelined chain instead of serial DVE work.
"""

import numpy as np
import ml_dtypes

import concourse.bass as bass
import concourse.mybir as mybir
import concourse.tile as tile
from concourse.bass_utils import run_bass_kernel_spmd

N_CORES = 8
N_TOKENS = 65536
D_IN = 1024
D_OUT = 2048
N_EXPERTS = 16
E_PER_CORE = N_EXPERTS // N_CORES        # 2
TOK_PER_CORE = N_TOKENS // N_CORES       # 8192
TOK_PER_EXPERT = N_TOKENS // N_EXPERTS   # 4096

P = 128
KB = D_IN // P            # 8 contraction blocks of 128
KP = KB // 2              # 4 DoubleRow k-pairs (256 contraction each)
TS = 512                  # token stripe
OB = 512                  # out-feature block (one PSUM bank)
N_OB = D_OUT // OB        # 4
N_TSUB = TS // P          # 4
STRIPES_PER_EXPERT = TOK_PER_EXPERT // TS  # 8
N_STRIPES = E_PER_CORE * STRIPES_PER_EXPERT  # 16

S_X = 4.0                 # keeps x (std 1) in e4m3 normal range
S_W = 128.0               # keeps W (std ~0.0054) out of e4m3 subnormals
OUT_SCALE = 1.0 / (S_X * S_W)

N_WARMUP_MM = 8

F32 = mybir.dt.float32
F16 = mybir.dt.float16
F8 = mybir.dt.float8e4
NP_F8 = ml_dtypes.float8_e4m3

DR = mybir.MatmulPerfMode.DoubleRow
SILU = mybir.ActivationFunctionType.Silu
ADD = mybir.AluOpType.add


def _split_multi_waits(nc):
    """This container's walrus encodes at most ONE sync-wait per instruction;
    hoist extras onto single-wait NoOps inserted just before, same engine."""
    for fn in nc.m.functions:
        for bb in fn.blocks:
            insts = list(bb.instructions)
            out = []
            dirty = False
            for inst in insts:
                si = inst.sync_info
                waits = list(si.on_wait) if si and si.on_wait else []
                if len(waits) > 1:
                    dirty = True
                    for j, w in enumerate(waits[:-1]):
                        nop = mybir.InstNoOp(
                            name=f"{inst.name}-prewait{j}", ins=[], outs=[]
                        )
                        nop.engine = inst.engine
                        nop.sync_info = mybir.SyncInfo(on_wait=[w], on_update=[])
                        out.append(nop)
                    inst.sync_info = mybir.SyncInfo(
                        on_wait=[waits[-1]],
                        on_update=list(si.on_update) if si.on_update else [],
                    )
                out.append(inst)
            if dirty:
                bb.instructions = out


def build_kernel():
    nc = bass.Bass()
    # xt pre-striped on host: [stripe, partition, kb, t] -> 4KB lines
    xt = nc.dram_tensor("xt", [N_STRIPES, P, KB, TS], F8, kind="ExternalInput")
    # w pre-laid per k-pair in SBUF tile order -> 4KB lines
    w = nc.dram_tensor("w", [E_PER_CORE, KP, P, 2 * D_OUT], F8,
                       kind="ExternalInput")
    bb = nc.dram_tensor("bb", [E_PER_CORE, P, D_OUT], F16, kind="ExternalInput")
    y = nc.dram_tensor("y", [TOK_PER_CORE, D_OUT], F16, kind="ExternalOutput")

    with tile.TileContext(nc) as tc:
        with (
            tc.tile_pool(name="persist", bufs=1) as persist,
            tc.tile_pool(name="xp", bufs=3) as xp,
            tc.tile_pool(name="outs", bufs=3) as outs,
            tc.tile_pool(name="psum", bufs=2, space="PSUM") as psump,
        ):
            # --- PE warm-up: matmuls over zeroed scratch, no DMA deps ---
            zs = persist.tile([P, 2, TS], F8, name="warm_src")
            nc.vector.memset(zs[:], 0.0)
            ps_warm = psump.tile([P, N_OB, OB], F32, tag="ps", name="ps_warm")
            for i in range(N_WARMUP_MM):
                nc.tensor.matmul(
                    ps_warm[:, i % N_OB, :],
                    lhsT=zs[:, :, 0:P],
                    rhs=zs[:],
                    start=True, stop=True,
                    perf_mode=DR,
                    skip_group_check=True,
                )

            # --- critical preload: expert-0 weights + x stripe 0 ---
            x8_tiles = {}
            x8_tiles[0] = xp.tile([P, KB, TS], F8, tag="x8", name="x8_s0")

            b_sb = [
                persist.tile([P, N_OB, OB], F16, name=f"bias_{e}")
                for e in range(E_PER_CORE)
            ]
            w8 = [
                [
                    persist.tile([P, 2, D_OUT], F8, name=f"w8_{e}_{h}")
                    for h in range(KP)
                ]
                for e in range(E_PER_CORE)
            ]

            def load_w(e, h, eng):
                eng.dma_start(w8[e][h][:], w[e, h].rearrange("p (j o) -> p j o",
                                                            j=2))

            load_w(0, 0, nc.scalar)
            nc.sync.dma_start(x8_tiles[0][:], xt[0])
            load_w(0, 1, nc.scalar)
            load_w(0, 2, nc.sync)
            load_w(0, 3, nc.sync)
            # bias-e0 deferred: the tiny gpsimd copy waits (in-order
            # sequencer) until w-e0 kp0 has landed, keeping the first part of
            # the head's queue slots for the critical weight stream while
            # still unblocking the stripe-0 drains promptly.
            bias_gate = persist.tile([P, 64], F8, name="bias_gate")
            nc.gpsimd.tensor_copy(bias_gate[:], w8[0][0][:, 0, 0:64])
            nc.gpsimd.dma_start(b_sb[0][:], bb[0])

            def load_expert(e):
                for h in range(KP):
                    eng = nc.scalar if h % 2 == 0 else nc.sync
                    load_w(e, h, eng)
                nc.gpsimd.dma_start(b_sb[e][:], bb[e])

            for e in range(E_PER_CORE):
                for s in range(STRIPES_PER_EXPERT):
                    g = e * STRIPES_PER_EXPERT + s
                    t0 = g * TS
                    if g in x8_tiles:
                        x8t = x8_tiles[g]
                    else:
                        x8t = xp.tile([P, KB, TS], F8, tag="x8", name="x8")
                        nc.sync.dma_start(x8t[:], xt[g])

                    def lhsT_of(kp, tsub):
                        return x8t[:, 2 * kp:2 * kp + 2, tsub * P:(tsub + 1) * P]

                    for tsub in range(N_TSUB):
                        last = g == N_STRIPES - 1 and tsub == N_TSUB - 1
                        rows = slice(t0 + tsub * P, t0 + (tsub + 1) * P)
                        ps = psump.tile([P, N_OB, OB], F32, tag="ps", name="ps")
                        # final tile runs ob-outer so each bank's accumulation
                        # group closes early and its quarter drain can start
                        # while later banks still accumulate
                        loops = (
                            [(kp, ob) for kp in range(KP) for ob in range(N_OB)]
                            if not last else
                            [(kp, ob) for ob in range(N_OB) for kp in range(KP)]
                        )
                        for kp, ob in loops:
                            nc.tensor.matmul(
                                ps[:, ob, :],
                                lhsT=lhsT_of(kp, tsub),
                                rhs=w8[e][kp][:, :, ob * OB:(ob + 1) * OB],
                                start=(kp == 0),
                                stop=(kp == KP - 1),
                                perf_mode=DR,
                            )
                        if not last:
                            y_sb = outs.tile([P, N_OB, OB], F32, tag="ysb",
                                             name="ysb")
                            nc.vector.tensor_tensor(y_sb[:], ps[:], b_sb[e][:],
                                                    ADD)
                            y_act = outs.tile([P, N_OB, OB], F16, tag="yact",
                                              name="yact")
                            nc.scalar.activation(y_act[:], y_sb[:], SILU,
                                                 scale=OUT_SCALE)
                            nc.scalar.dma_start(y[rows, :], y_act[:])
                        else:
                            # final tile: per-ob pipelined drain so the tail
                            # is DVE->ACT->small store chains, not one big op
                            y_act = outs.tile([P, N_OB, OB], F16, tag="yact",
                                              name="yact_f")
                            for ob in range(N_OB):
                                y_sb = outs.tile([P, OB], F32, tag="ysbq",
                                                 name="ysbq")
                                nc.vector.tensor_tensor(
                                    y_sb[:], ps[:, ob, :], b_sb[e][:, ob, :],
                                    ADD)
                                nc.scalar.activation(
                                    y_act[:, ob, :], y_sb[:], SILU,
                                    scale=OUT_SCALE)
                                # all final stores on the now-idle sync ring:
                                # keeps the ACT sequencer's silu chain free
                                # of 600ns store-issue slices
                                nc.sync.dma_start(
                                    y[rows, ob * OB:(ob + 1) * OB],
                                    y_act[:, ob, :])
                    if g == 0:
                        load_expert(1)

    _split_multi_waits(nc)
    return nc


_NC_CACHE = None


def _get_nc():
    global _NC_CACHE
    if _NC_CACHE is None:
        _NC_CACHE = build_kernel()
    return _NC_CACHE


def _in_maps(sorted_features, routing_matrix, routing_bias):
    maps = []
    for c in range(N_CORES):
        rows = slice(c * TOK_PER_CORE, (c + 1) * TOK_PER_CORE)
        es = slice(c * E_PER_CORE, (c + 1) * E_PER_CORE)
        # [stripe, partition, kb, t]: element (s,p,kb,t) = S_X*X_c[s*TS+t, kb*P+p]
        xt_c = np.ascontiguousarray(
            (sorted_features[rows] * S_X)
            .reshape(N_STRIPES, TS, KB, P)
            .transpose(0, 3, 2, 1)
            .astype(NP_F8)
        )
        # [e, kp, p, j*D_OUT+o] = S_W * W_e[(2*kp+j)*128+p, o]
        w_c = np.ascontiguousarray(
            (routing_matrix[:, :, es] * S_W)
            .transpose(2, 0, 1)                      # [E, D_IN, D_OUT]
            .reshape(E_PER_CORE, KP, 2, P, D_OUT)
            .transpose(0, 1, 3, 2, 4)                # [E, KP, P, 2, D_OUT]
            .reshape(E_PER_CORE, KP, P, 2 * D_OUT)
            .astype(NP_F8)
        )
        b_c = np.ascontiguousarray(
            np.broadcast_to(
                (routing_bias[:, es] * (S_X * S_W)).T[:, None, :],
                (E_PER_CORE, P, D_OUT),
            ).astype(np.float16)
        )
        maps.append({"xt": xt_c, "w": w_c, "bb": b_c})
    return maps


def run(sorted_features, routing_matrix, routing_bias, **run_kwargs):
    nc = _get_nc()
    maps = _in_maps(sorted_features, routing_matrix, routing_bias)
    res = run_bass_kernel_spmd(nc, maps, core_ids=list(range(N_CORES)), **run_kwargs)
    out = np.concatenate(
        [np.asarray(res.results[c]["y"]) for c in range(N_CORES)], axis=0
    ).astype(np.float32)
    return out, res


def kernel(sorted_features, expert_ids_sorted, routing_matrix, routing_bias):
    assert sorted_features.shape == (N_TOKENS, D_IN)
    assert routing_matrix.shape == (D_IN, D_OUT, N_EXPERTS)
    assert routing_bias.shape == (D_OUT, N_EXPERTS)
    out, _ = run(
        np.asarray(sorted_features, dtype=np.float32),
        np.asarray(routing_matrix, dtype=np.float32),
        np.asarray(routing_bias, dtype=np.float32),
    )
    return out



# revision 4
# speedup vs baseline: 1.0152x; 1.0152x over previous
"""MoE expert-collection grouped GEMM for Trainium2, expert-parallel over 8
NeuronCores, fp8 DoubleRow matmuls, weight-stationary / transposed output.

Problem (hardcoded shapes):
  sorted_features  [65536, 1024] f32   tokens sorted by expert, 4096/expert
  expert_ids_sorted[65536] i32         unused: split is static equal-count
  routing_matrix   [1024, 2048, 16] f32
  routing_bias     [2048, 16] f32
  out = silu(x_e @ W_e + b_e) per expert  -> [65536, 2048] f32

Sharding: expert-parallel, 2 experts (= 8192 contiguous sorted tokens) per
core.

Design (v2, weight-stationary): each matmul computes a [128 outs, 512 toks]
PSUM tile: lhsT = w chunk [128, 2(DR), 128 outs], rhs = xT chunk
[128, 2(DR), 512 toks], accumulated over 4 k-pairs.  The output is produced
TRANSPOSED (yt [2048, 8192] f16) and de-transposed on the host.  This makes
the bias per-PARTITION, so the whole PSUM drain is ONE scalar-engine
activation: silu(psum * OUT_SCALE + bias_fp32) reading PSUM directly --
there is no DVE tensor_tensor at all, which removes the PSUM-drain
bottleneck of the token-stationary layout.

Head shaping: the first block runs stripe 0 with only out-blocks 0-7 so the
critical preload is w[0..8) (1MB) + x stripe-0 (512KB) instead of the full
2MB expert; stripe-0's x is split in 4 per-kp chunk DMAs so the first
matmul only waits for 256KB.  Out-blocks 8-15 of stripe 0 run as a later
block against the still-resident x chunks.  Weight/x loads ride the sync
ring in need order (first 8 w tiles + bias-0 on the then-idle scalar ring);
y stores ride the scalar ring at 4-out-block granularity except the final
two, which are per-out-block on the then-idle sync ring to shorten the
tail.  6 zero-matmul warmups flip the PE HAM clock-gate while the preload
streams.
"""

import numpy as np
import ml_dtypes

import concourse.bass as bass
import concourse.mybir as mybir
import concourse.tile as tile
from concourse.bass_utils import run_bass_kernel_spmd

N_CORES = 8
N_TOKENS = 65536
D_IN = 1024
D_OUT = 2048
N_EXPERTS = 16
E_PER_CORE = N_EXPERTS // N_CORES        # 2
TOK_PER_CORE = N_TOKENS // N_CORES       # 8192
TOK_PER_EXPERT = N_TOKENS // N_EXPERTS   # 4096

P = 128
KP = 4                     # DoubleRow k-pairs (256 contraction each)
TS = 512                   # token stripe (matmul moving free dim)
N_STRIPES = TOK_PER_CORE // TS           # 16
OBW = 128                  # out-feature block (psum partition dim)
N_OB = D_OUT // OBW        # 16

S_X = 4.0                  # keeps x (std 1) in e4m3 normal range
S_W = 128.0                # keeps W (std ~0.0054) out of e4m3 subnormals
OUT_SCALE = 1.0 / (S_X * S_W)

N_WARMUP_MM = 6

F32 = mybir.dt.float32
F16 = mybir.dt.float16
F8 = mybir.dt.float8e4
NP_F8 = ml_dtypes.float8_e4m3

DR = mybir.MatmulPerfMode.DoubleRow
SILU = mybir.ActivationFunctionType.Silu


def _split_multi_waits(nc):
    """This container's walrus encodes at most ONE sync-wait per instruction;
    hoist extras onto single-wait NoOps inserted just before, same engine."""
    for fn in nc.m.functions:
        for bb in fn.blocks:
            insts = list(bb.instructions)
            out = []
            dirty = False
            for inst in insts:
                si = inst.sync_info
                waits = list(si.on_wait) if si and si.on_wait else []
                if len(waits) > 1:
                    dirty = True
                    for j, w in enumerate(waits[:-1]):
                        nop = mybir.InstNoOp(
                            name=f"{inst.name}-prewait{j}", ins=[], outs=[]
                        )
                        nop.engine = inst.engine
                        nop.sync_info = mybir.SyncInfo(on_wait=[w], on_update=[])
                        out.append(nop)
                    inst.sync_info = mybir.SyncInfo(
                        on_wait=[waits[-1]],
                        on_update=list(si.on_update) if si.on_update else [],
                    )
                out.append(inst)
            if dirty:
                bb.instructions = out


def build_kernel():
    nc = bass.Bass()
    # xt[s, kp, p, j*TS+t] = S_X * X[s*TS+t, kp*256 + j*128 + p]
    xt = nc.dram_tensor("xt", [N_STRIPES, KP, P, 2 * TS], F8,
                        kind="ExternalInput")
    # w[e, ob, p, kp*256 + j*128 + i] = S_W * W_e[kp*256 + j*128 + p, ob*128+i]
    w = nc.dram_tensor("w", [E_PER_CORE, N_OB, P, KP * 2 * OBW], F8,
                       kind="ExternalInput")
    # bb[e, p, ob] = bias[ob*128 + p] (exact fp32, applied inside ACT)
    bb = nc.dram_tensor("bb", [E_PER_CORE, P, N_OB], F32, kind="ExternalInput")
    # transposed output; host does yt.T
    yt = nc.dram_tensor("yt", [D_OUT, TOK_PER_CORE], F16, kind="ExternalOutput")

    # block schedule: (expert, [stripe ids], ob_lo, ob_hi)
    # stripe ids are global per-core (0..15); expert e covers stripes 8e..8e+8
    blocks = [
        (0, [0], 0, 8),         # head: small critical preload
        (0, [1], 0, 16),
        (0, [0], 8, 16),        # finish stripe 0 against resident x
        (0, [2, 3], 0, 16),
        (0, [4, 5], 0, 16),
        (0, [6, 7], 0, 16),
        (1, [8, 9], 0, 16),
        (1, [10, 11], 0, 16),
        (1, [12, 13], 0, 16),
        (1, [14], 0, 16),
        (1, [15], 0, 16),       # tail: single stripe, small final drains
    ]

    with tile.TileContext(nc) as tc:
        with (
            tc.tile_pool(name="persist", bufs=1) as persist,
            tc.tile_pool(name="xs", bufs=6) as xsp,
            tc.tile_pool(name="outs", bufs=3) as outs,
            tc.tile_pool(name="psum", bufs=6, space="PSUM") as psump,
            tc.tile_pool(name="psumw", bufs=1, space="PSUM") as psumw,
        ):
            # --- PE warm-up: matmuls over zeroed scratch, no DMA deps.
            # N=256 so the warmup chain finishes before the first real
            # matmul's operands land (~8.9us) instead of queueing ahead.
            zs = persist.tile([P, 2, 256], F8, name="warm_src")
            nc.vector.memset(zs[:], 0.0)
            ps_warm = psumw.tile([P, 256], F32, name="ps_warm")
            for i in range(N_WARMUP_MM):
                nc.tensor.matmul(
                    ps_warm[:],
                    lhsT=zs[:, :, 0:P],
                    rhs=zs[:],
                    start=True, stop=True,
                    perf_mode=DR,
                    skip_group_check=True,
                )

            # --- persistent weight/bias tiles ---
            # e0 ob0..7: single-ob tiles (fine-grained head preload)
            w8s = [persist.tile([P, KP, 2, OBW], F8, name=f"w8s_{ob}")
                   for ob in range(8)]
            # remaining obs: double-ob tiles [P, 2, KP, 2, OBW]
            w8d = {}
            for e in range(E_PER_CORE):
                q0 = 4 if e == 0 else 0
                for q in range(q0, N_OB // 2):
                    w8d[(e, q)] = persist.tile([P, 2, KP, 2, OBW], F8,
                                               name=f"w8d_{e}_{q}")
            b_sb = [persist.tile([P, N_OB], F32, name=f"bias_{e}")
                    for e in range(E_PER_CORE)]

            def w_ap(e, ob, kp):
                if e == 0 and ob < 8:
                    return w8s[ob][:, kp, :, :]
                return w8d[(e, ob // 2)][:, ob % 2, kp, :, :]

            def load_w_single(ob, eng):
                eng.dma_start(
                    w8s[ob][:],
                    w[0, ob].rearrange("p (k j i) -> p k j i", k=KP, j=2))

            def load_w_double(e, q, eng):
                eng.dma_start(
                    w8d[(e, q)][:],
                    w[e, 2 * q:2 * q + 2].rearrange(
                        "o p (k j i) -> p o k j i", k=KP, j=2))

            # x tiles: stripe 0 = 4 per-kp chunks (first-matmul gating);
            # all other stripes = one [P, KP, 2, TS] tile
            x0c = [xsp.tile([P, 2, TS], F8, tag="x0", name=f"x0_{kp}")
                   for kp in range(KP)]
            x_tiles = {}

            def load_x0_chunk(kp):
                nc.sync.dma_start(
                    x0c[kp][:], xt[0, kp].rearrange("p (j t) -> p j t", j=2))

            def load_stripe(s):
                x_tiles[s] = xsp.tile([P, KP, 2, TS], F8, tag="xs",
                                      name=f"xs_{s}")
                nc.sync.dma_start(
                    x_tiles[s][:],
                    xt[s].rearrange("k p (j t) -> p k j t", j=2))

            def x_ap(s, kp):
                if s == 0:
                    return x0c[kp][:]
                return x_tiles[s][:, kp, :, :]

            # --- head preload, need-ordered ---
            # scalar ring: bias0 + first 8 single-ob weight tiles
            nc.scalar.dma_start(b_sb[0][:], bb[0])
            for ob in range(8):
                load_w_single(ob, nc.scalar)
            # sync ring: stripe-0 chunks, then e0 upper weights + stripe 1
            for kp in range(KP):
                load_x0_chunk(kp)
            load_w_double(0, 4, nc.sync)
            load_w_double(0, 5, nc.sync)
            load_stripe(1)
            load_w_double(0, 6, nc.sync)
            load_w_double(0, 7, nc.sync)
            # bias e1 on the gpsimd software queue (tiny, needed late)
            nc.gpsimd.dma_start(b_sb[1][:], bb[1])

            # x/w prefetch emitted at the start of block bi (sync ring)
            prefetch = {
                1: [lambda: load_stripe(2), lambda: load_stripe(3)],
                2: [lambda: load_stripe(4), lambda: load_stripe(5)],
                3: [lambda: load_stripe(6), lambda: load_stripe(7),
                    lambda: load_w_double(1, 0, nc.sync),
                    lambda: load_w_double(1, 1, nc.sync)],
                4: [lambda: load_stripe(8), lambda: load_stripe(9),
                    lambda: load_w_double(1, 2, nc.sync),
                    lambda: load_w_double(1, 3, nc.sync)],
                5: [lambda: load_stripe(10), lambda: load_stripe(11),
                    lambda: load_w_double(1, 4, nc.sync),
                    lambda: load_w_double(1, 5, nc.sync)],
                6: [lambda: load_stripe(12), lambda: load_stripe(13),
                    lambda: load_w_double(1, 6, nc.sync),
                    lambda: load_w_double(1, 7, nc.sync)],
                7: [lambda: load_stripe(14), lambda: load_stripe(15)],
            }

            n_blocks = len(blocks)
            for bi, (e, stripes, ob_lo, ob_hi) in enumerate(blocks):
                for fn in prefetch.get(bi, []):
                    fn()
                span = len(stripes) * TS
                last_block = bi == n_blocks - 1
                for og in range(ob_lo, ob_hi, 4):
                    obs = list(range(og, min(og + 4, ob_hi)))
                    tail_og = last_block and og + 4 >= ob_hi
                    if not tail_og:
                        tag = "ytp" if span == 2 * TS else "yts"
                        y4 = outs.tile([P, 4, span], F16, tag=tag, name="y4")
                    for oi, ob in enumerate(obs):
                        tail_ob = tail_og
                        pss = [psump.tile([P, TS], F32, tag="ps", name="ps")
                               for _ in stripes]
                        for kp in range(KP):
                            for si in range(len(stripes)):
                                nc.tensor.matmul(
                                    pss[si][:],
                                    lhsT=w_ap(e, ob, kp),
                                    rhs=x_ap(stripes[si], kp),
                                    start=(kp == 0),
                                    stop=(kp == KP - 1),
                                    perf_mode=DR,
                                )
                        for si, s in enumerate(stripes):
                            if tail_ob:
                                # final two obs: own drain + store on the
                                # now-idle sync ring for a short tail
                                y1 = outs.tile([P, TS], F16, tag="ytt",
                                               name="y1")
                                nc.scalar.activation(
                                    y1[:], pss[si][:], SILU,
                                    bias=b_sb[e][:, ob:ob + 1],
                                    scale=OUT_SCALE)
                                nc.sync.dma_start(
                                    yt[ob * OBW:(ob + 1) * OBW,
                                       s * TS:(s + 1) * TS],
                                    y1[:])
                            else:
                                nc.scalar.activation(
                                    y4[:, oi, si * TS:(si + 1) * TS],
                                    pss[si][:], SILU,
                                    bias=b_sb[e][:, ob:ob + 1],
                                    scale=OUT_SCALE)
                    if not tail_og:
                        t0 = stripes[0] * TS
                        dst = yt[og * OBW:(og + 4) * OBW,
                                 t0:t0 + span].rearrange(
                                     "(o p) t -> p o t", p=P)
                        nc.scalar.dma_start(dst, y4[:])

    _split_multi_waits(nc)
    return nc


_NC_CACHE = None


def _get_nc():
    global _NC_CACHE
    if _NC_CACHE is None:
        _NC_CACHE = build_kernel()
    return _NC_CACHE


def _in_maps(sorted_features, routing_matrix, routing_bias):
    maps = []
    for c in range(N_CORES):
        rows = slice(c * TOK_PER_CORE, (c + 1) * TOK_PER_CORE)
        es = slice(c * E_PER_CORE, (c + 1) * E_PER_CORE)
        # [s, kp, p, j*TS+t] = S_X * X[s*TS+t, kp*256 + j*128 + p]
        xt_c = np.ascontiguousarray(
            (sorted_features[rows] * S_X)
            .astype(NP_F8)
            .reshape(N_STRIPES, TS, KP, 2, P)   # [s, t, kp, j, p]
            .transpose(0, 2, 4, 3, 1)           # [s, kp, p, j, t]
            .reshape(N_STRIPES, KP, P, 2 * TS)
        )
        # [e, ob, p, kp*256 + j*128 + i]
        w_c = np.ascontiguousarray(
            (routing_matrix[:, :, es] * S_W)
            .astype(NP_F8)
            .transpose(2, 0, 1)                      # [e, d_in, d_out]
            .reshape(E_PER_CORE, KP, 2, P, N_OB, OBW)  # [e,kp,j,p,ob,i]
            .transpose(0, 4, 3, 1, 2, 5)             # [e, ob, p, kp, j, i]
            .reshape(E_PER_CORE, N_OB, P, KP * 2 * OBW)
        )
        # [e, p, ob] = bias[ob*128 + p]  (exact fp32)
        b_c = np.ascontiguousarray(
            routing_bias[:, es].T                    # [e, d_out]
            .reshape(E_PER_CORE, N_OB, P)
            .transpose(0, 2, 1)
            .astype(np.float32)
        )
        maps.append({"xt": xt_c, "w": w_c, "bb": b_c})
    return maps


def run(sorted_features, routing_matrix, routing_bias, **run_kwargs):
    nc = _get_nc()
    maps = _in_maps(sorted_features, routing_matrix, routing_bias)
    res = run_bass_kernel_spmd(nc, maps, core_ids=list(range(N_CORES)),
                               **run_kwargs)
    out = np.empty((N_TOKENS, D_OUT), dtype=np.float32)
    for c in range(N_CORES):
        yt_c = np.asarray(res.results[c]["yt"])
        out[c * TOK_PER_CORE:(c + 1) * TOK_PER_CORE] = yt_c.T
    return out, res


def kernel(sorted_features, expert_ids_sorted, routing_matrix, routing_bias):
    assert sorted_features.shape == (N_TOKENS, D_IN)
    assert routing_matrix.shape == (D_IN, D_OUT, N_EXPERTS)
    assert routing_bias.shape == (D_OUT, N_EXPERTS)
    out, _ = run(
        np.asarray(sorted_features, dtype=np.float32),
        np.asarray(routing_matrix, dtype=np.float32),
        np.asarray(routing_bias, dtype=np.float32),
    )
    return out


# revision 6
# speedup vs baseline: 1.0268x; 1.0115x over previous
"""MoE expert-collection grouped GEMM for Trainium2, expert-parallel over 8
NeuronCores, fp8 DoubleRow matmuls, weight-stationary / transposed output.

Problem (hardcoded shapes):
  sorted_features  [65536, 1024] f32   tokens sorted by expert, 4096/expert
  expert_ids_sorted[65536] i32         unused: split is static equal-count
  routing_matrix   [1024, 2048, 16] f32
  routing_bias     [2048, 16] f32
  out = silu(x_e @ W_e + b_e) per expert  -> [65536, 2048] f32

Sharding: expert-parallel, 2 experts (= 8192 contiguous sorted tokens) per
core.

Design (weight-stationary): each matmul computes a [128 outs, 512 toks]
PSUM tile: lhsT = w chunk [128, 2(DR), 128 outs], rhs = xT chunk
[128, 2(DR), 512 toks], accumulated over 4 k-pairs.  The output is produced
TRANSPOSED (yt [2048, 8192] f16) and de-transposed on the host.  This makes
the bias per-PARTITION, so the whole PSUM drain is ONE scalar-engine
activation: silu(psum * OUT_SCALE + bias_fp32) reading PSUM directly --
no DVE work at all.  Tokens are processed in stripe-PAIR blocks (1024
tokens) so one [128, 2, 512] two-bank ACT drains a whole ob, keeping the
scalar engine under ~80% busy; y stores ride the sync ring (the scalar
ring is ACT-only) at 4-ob granularity (2KB DRAM runs).

Head shaping: stripe 0 runs first against only out-blocks 0-7 so the
critical preload is half the expert's weights; out-blocks 8-15 of stripe 0
run as a third block against the still-resident x.  Head DMAs are few and
large (2-4KB per-partition lines) because walrus shares completion
semaphores across queues -- many small head DMAs serialize on sem reuse.
6 zero-matmul warmups flip the PE HAM clock-gate while the preload
streams.  The final block's last out-blocks drain per-stripe with small
sync-ring stores to shorten the tail.
"""

import numpy as np
import ml_dtypes

import concourse.bass as bass
import concourse.mybir as mybir
import concourse.tile as tile
from concourse.bass_utils import run_bass_kernel_spmd

N_CORES = 8
N_TOKENS = 65536
D_IN = 1024
D_OUT = 2048
N_EXPERTS = 16
E_PER_CORE = N_EXPERTS // N_CORES        # 2
TOK_PER_CORE = N_TOKENS // N_CORES       # 8192
TOK_PER_EXPERT = N_TOKENS // N_EXPERTS   # 4096

P = 128
KP = 4                     # DoubleRow k-pairs (256 contraction each)
TS = 512                   # token stripe (matmul moving free dim)
N_STRIPES = TOK_PER_CORE // TS           # 16
OBW = 128                  # out-feature block (psum partition dim)
N_OB = D_OUT // OBW        # 16

S_X = 4.0                  # keeps x (std 1) in e4m3 normal range
S_W = 128.0                # keeps W (std ~0.0054) out of e4m3 subnormals
OUT_SCALE = 1.0 / (S_X * S_W)

N_WARMUP_MM = 6

F32 = mybir.dt.float32
F16 = mybir.dt.float16
F8 = mybir.dt.float8e4
NP_F8 = ml_dtypes.float8_e4m3

DR = mybir.MatmulPerfMode.DoubleRow
SILU = mybir.ActivationFunctionType.Silu


def _split_multi_waits(nc):
    """This container's walrus encodes at most ONE sync-wait per instruction;
    hoist extras onto single-wait NoOps inserted just before, same engine."""
    for fn in nc.m.functions:
        for bb in fn.blocks:
            insts = list(bb.instructions)
            out = []
            dirty = False
            for inst in insts:
                si = inst.sync_info
                waits = list(si.on_wait) if si and si.on_wait else []
                if len(waits) > 1:
                    dirty = True
                    for j, w in enumerate(waits[:-1]):
                        nop = mybir.InstNoOp(
                            name=f"{inst.name}-prewait{j}", ins=[], outs=[]
                        )
                        nop.engine = inst.engine
                        nop.sync_info = mybir.SyncInfo(on_wait=[w], on_update=[])
                        out.append(nop)
                    inst.sync_info = mybir.SyncInfo(
                        on_wait=[waits[-1]],
                        on_update=list(si.on_update) if si.on_update else [],
                    )
                out.append(inst)
            if dirty:
                bb.instructions = out


def build_kernel():
    nc = bass.Bass()
    # xt[s, kp, p, j*TS+t] = S_X * X[s*TS+t, kp*256 + j*128 + p]
    xt = nc.dram_tensor("xt", [N_STRIPES, KP, P, 2 * TS], F8,
                        kind="ExternalInput")
    # w[e, ob, p, kp*256 + j*128 + i] = S_W * W_e[kp*256 + j*128 + p, ob*128+i]
    w = nc.dram_tensor("w", [E_PER_CORE, N_OB, P, KP * 2 * OBW], F8,
                       kind="ExternalInput")
    # bb[e, p, ob] = bias[ob*128 + p] (exact fp32, applied inside ACT)
    bb = nc.dram_tensor("bb", [E_PER_CORE, P, N_OB], F32, kind="ExternalInput")
    # transposed output; host does yt.T
    yt = nc.dram_tensor("yt", [D_OUT, TOK_PER_CORE], F16, kind="ExternalOutput")

    # block schedule: (expert, [stripe ids], ob_lo, ob_hi)
    blocks = [
        (0, [0], 0, 8),         # head: small critical preload
        (0, [1], 0, 16),
        (0, [0], 8, 16),        # finish stripe 0 against resident x
        (0, [2, 3], 0, 16),
        (0, [4, 5], 0, 16),
        (0, [6, 7], 0, 16),
        (1, [8, 9], 0, 16),
        (1, [10, 11], 0, 16),
        (1, [12, 13], 0, 16),
        (1, [14, 15], 0, 16),   # tail pair; last obs drain per-stripe
    ]

    with tile.TileContext(nc) as tc:
        with (
            tc.tile_pool(name="persist", bufs=1) as persist,
            tc.tile_pool(name="xs", bufs=5) as xsp,
            tc.tile_pool(name="outs", bufs=4) as outs,
            tc.tile_pool(name="psum", bufs=3, space="PSUM") as psump,
            tc.tile_pool(name="psum1", bufs=2, space="PSUM") as psump1,
        ):
            # --- PE warm-up: matmuls over zeroed scratch, no DMA deps.
            # N=256 so the chain finishes before the first real matmul's
            # operands land instead of queueing ahead of it.
            zs = persist.tile([P, 2, 256], F8, name="warm_src")
            nc.vector.memset(zs[:], 0.0)
            ps_warm = psump1.tile([P, TS], F32, tag="ps1", name="ps_warm")
            for i in range(N_WARMUP_MM):
                nc.tensor.matmul(
                    ps_warm[:, 0:256],
                    lhsT=zs[:, :, 0:P],
                    rhs=zs[:],
                    start=True, stop=True,
                    perf_mode=DR,
                    skip_group_check=True,
                )

            # --- persistent weight/bias tiles ---
            # e0: obs 0-7 as 2-ob tiles (2KB lines, fine head granularity),
            # obs 8-15 and all of e1 as 4-ob tiles (4KB lines).
            w8d = {q: persist.tile([P, 2, KP, 2, OBW], F8, name=f"w8d_{q}")
                   for q in range(4)}
            w8q = {}
            for e in range(E_PER_CORE):
                q0 = 2 if e == 0 else 0
                for q in range(q0, 4):
                    w8q[(e, q)] = persist.tile([P, 4, KP, 2, OBW], F8,
                                               name=f"w8q_{e}_{q}")
            b_sb = [persist.tile([P, N_OB], F32, name=f"bias_{e}")
                    for e in range(E_PER_CORE)]

            def w_ap(e, ob, kp):
                if e == 0 and ob < 8:
                    return w8d[ob // 2][:, ob % 2, kp, :, :]
                return w8q[(e, ob // 4)][:, ob % 4, kp, :, :]

            def load_w2(q, eng):
                eng.dma_start(
                    w8d[q][:],
                    w[0, 2 * q:2 * q + 2].rearrange(
                        "o p (k j i) -> p o k j i", k=KP, j=2))

            def load_w4(e, q, eng):
                eng.dma_start(
                    w8q[(e, q)][:],
                    w[e, 4 * q:4 * q + 4].rearrange(
                        "o p (k j i) -> p o k j i", k=KP, j=2))

            # x tiles: stripes 0/1 as half-stripe (2 k-pair) tiles for head
            # granularity; the rest as full-stripe tiles (4KB lines).
            xh = {}
            x_tiles = {}

            def load_xhalf(s, h):
                xh[(s, h)] = xsp.tile([P, 2, 2, TS], F8, tag="xh",
                                      name=f"xh_{s}_{h}")
                nc.sync.dma_start(
                    xh[(s, h)][:],
                    xt[s, 2 * h:2 * h + 2].rearrange(
                        "k p (j t) -> p k j t", j=2))

            def load_stripe(s):
                x_tiles[s] = xsp.tile([P, KP, 2, TS], F8, tag="xs",
                                      name=f"xs_{s}")
                nc.sync.dma_start(
                    x_tiles[s][:],
                    xt[s].rearrange("k p (j t) -> p k j t", j=2))

            def x_ap(s, kp):
                if s in (0, 1):
                    return xh[(s, kp // 2)][:, kp % 2, :, :]
                return x_tiles[s][:, kp, :, :]

            # --- head preload, need-ordered, few+large DMAs ---
            # sync ring: first weights + stripe-0/1 halves
            load_w2(0, nc.sync)          # obs 0-1
            load_xhalf(0, 0)             # stripe 0, kp 0-1
            load_xhalf(0, 1)
            load_w2(1, nc.sync)          # obs 2-3
            # scalar ring (ACT is idle until ~12us): bias + obs 4-7
            nc.scalar.dma_start(b_sb[0][:], bb[0])
            load_w2(2, nc.scalar)
            load_w2(3, nc.scalar)
            # sync continues: stripe 1, e0 upper weights
            load_xhalf(1, 0)
            load_xhalf(1, 1)
            load_w4(0, 2, nc.sync)       # obs 8-11
            load_w4(0, 3, nc.sync)       # obs 12-15
            nc.gpsimd.dma_start(b_sb[1][:], bb[1])

            # x/w prefetch emitted on sync at the start of block bi
            prefetch = {
                1: [lambda: load_stripe(2), lambda: load_stripe(3)],
                2: [lambda: load_stripe(4), lambda: load_stripe(5)],
                3: [lambda: load_stripe(6), lambda: load_stripe(7),
                    lambda: load_w4(1, 0, nc.sync),
                    lambda: load_w4(1, 1, nc.sync)],
                4: [lambda: load_stripe(8), lambda: load_stripe(9),
                    lambda: load_w4(1, 2, nc.sync),
                    lambda: load_w4(1, 3, nc.sync)],
                5: [lambda: load_stripe(10), lambda: load_stripe(11)],
                6: [lambda: load_stripe(12), lambda: load_stripe(13)],
                7: [lambda: load_stripe(14), lambda: load_stripe(15)],
            }

            n_blocks = len(blocks)
            for bi, (e, stripes, ob_lo, ob_hi) in enumerate(blocks):
                for fn in prefetch.get(bi, []):
                    fn()
                pair = len(stripes) == 2
                span = len(stripes) * TS
                t0 = stripes[0] * TS
                last_block = bi == n_blocks - 1
                for og in range(ob_lo, ob_hi, 4):
                    obs = list(range(og, min(og + 4, ob_hi)))
                    tail_og = last_block and og + 4 >= ob_hi
                    if not tail_og:
                        tag = "ytp" if pair else "yts"
                        y4 = outs.tile([P, 4, span], F16, tag=tag, name="y4")
                    for oi, ob in enumerate(obs):
                        if pair:
                            ps = psump.tile([P, 2, TS], F32, tag="ps2",
                                            name="ps2")
                            ps_of = [ps[:, 0, :], ps[:, 1, :]]
                        else:
                            ps = psump1.tile([P, TS], F32, tag="ps1",
                                             name="ps1")
                            ps_of = [ps[:]]
                        for kp in range(KP):
                            for si in range(len(stripes)):
                                nc.tensor.matmul(
                                    ps_of[si],
                                    lhsT=w_ap(e, ob, kp),
                                    rhs=x_ap(stripes[si], kp),
                                    start=(kp == 0),
                                    stop=(kp == KP - 1),
                                    perf_mode=DR,
                                )
                        bias_ap = b_sb[e][:, ob:ob + 1]
                        if not tail_og:
                            # one ACT drains the whole ob (both banks)
                            y_dst = y4[:, oi, :]
                            if pair:
                                y_dst = y_dst.rearrange("p (s t) -> p s t",
                                                        s=2)
                            nc.scalar.activation(
                                y_dst, ps[:], SILU, bias=bias_ap,
                                scale=OUT_SCALE)
                        elif ob < ob_hi - 2:
                            # tail obs 12-13: per-ob drain + store
                            y1 = outs.tile([P, 2, TS], F16, tag="ytm",
                                           name="y1")
                            nc.scalar.activation(y1[:], ps[:], SILU,
                                                 bias=bias_ap,
                                                 scale=OUT_SCALE)
                            nc.sync.dma_start(
                                yt[ob * OBW:(ob + 1) * OBW, t0:t0 + span],
                                y1[:])
                        else:
                            # final two obs: per-stripe drains + stores so
                            # the post-last-matmul chain is short
                            for si, s in enumerate(stripes):
                                ys = outs.tile([P, TS], F16, tag="ytt",
                                               name="ys")
                                nc.scalar.activation(ys[:], ps_of[si], SILU,
                                                     bias=bias_ap,
                                                     scale=OUT_SCALE)
                                nc.sync.dma_start(
                                    yt[ob * OBW:(ob + 1) * OBW,
                                       s * TS:(s + 1) * TS],
                                    ys[:])
                    if not tail_og:
                        dst = yt[og * OBW:(og + 4) * OBW,
                                 t0:t0 + span].rearrange(
                                     "(o p) t -> p o t", p=P)
                        nc.sync.dma_start(dst, y4[:])

    _split_multi_waits(nc)
    return nc


_NC_CACHE = None


def _get_nc():
    global _NC_CACHE
    if _NC_CACHE is None:
        _NC_CACHE = build_kernel()
    return _NC_CACHE


def _in_maps(sorted_features, routing_matrix, routing_bias):
    maps = []
    for c in range(N_CORES):
        rows = slice(c * TOK_PER_CORE, (c + 1) * TOK_PER_CORE)
        es = slice(c * E_PER_CORE, (c + 1) * E_PER_CORE)
        # [s, kp, p, j*TS+t] = S_X * X[s*TS+t, kp*256 + j*128 + p]
        xt_c = np.ascontiguousarray(
            (sorted_features[rows] * S_X)
            .astype(NP_F8)
            .reshape(N_STRIPES, TS, KP, 2, P)   # [s, t, kp, j, p]
            .transpose(0, 2, 4, 3, 1)           # [s, kp, p, j, t]
            .reshape(N_STRIPES, KP, P, 2 * TS)
        )
        # [e, ob, p, kp*256 + j*128 + i]
        w_c = np.ascontiguousarray(
            (routing_matrix[:, :, es] * S_W)
            .astype(NP_F8)
            .transpose(2, 0, 1)                      # [e, d_in, d_out]
            .reshape(E_PER_CORE, KP, 2, P, N_OB, OBW)  # [e,kp,j,p,ob,i]
            .transpose(0, 4, 3, 1, 2, 5)             # [e, ob, p, kp, j, i]
            .reshape(E_PER_CORE, N_OB, P, KP * 2 * OBW)
        )
        # [e, p, ob] = bias[ob*128 + p]  (exact fp32)
        b_c = np.ascontiguousarray(
            routing_bias[:, es].T                    # [e, d_out]
            .reshape(E_PER_CORE, N_OB, P)
            .transpose(0, 2, 1)
            .astype(np.float32)
        )
        maps.append({"xt": xt_c, "w": w_c, "bb": b_c})
    return maps


def run(sorted_features, routing_matrix, routing_bias, **run_kwargs):
    nc = _get_nc()
    maps = _in_maps(sorted_features, routing_matrix, routing_bias)
    res = run_bass_kernel_spmd(nc, maps, core_ids=list(range(N_CORES)),
                               **run_kwargs)
    out = np.empty((N_TOKENS, D_OUT), dtype=np.float32)
    for c in range(N_CORES):
        yt_c = np.asarray(res.results[c]["yt"])
        out[c * TOK_PER_CORE:(c + 1) * TOK_PER_CORE] = yt_c.T
    return out, res


def kernel(sorted_features, expert_ids_sorted, routing_matrix, routing_bias):
    assert sorted_features.shape == (N_TOKENS, D_IN)
    assert routing_matrix.shape == (D_IN, D_OUT, N_EXPERTS)
    assert routing_bias.shape == (D_OUT, N_EXPERTS)
    out, _ = run(
        np.asarray(sorted_features, dtype=np.float32),
        np.asarray(routing_matrix, dtype=np.float32),
        np.asarray(routing_bias, dtype=np.float32),
    )
    return out


# revision 10
# speedup vs baseline: 1.0404x; 1.0132x over previous
"""MoE expert-collection grouped GEMM for Trainium2, expert-parallel over 8
NeuronCores, fp8 DoubleRow matmuls, weight-stationary / transposed output.

Problem (hardcoded shapes):
  sorted_features  [65536, 1024] f32   tokens sorted by expert, 4096/expert
  expert_ids_sorted[65536] i32         unused: split is static equal-count
  routing_matrix   [1024, 2048, 16] f32
  routing_bias     [2048, 16] f32
  out = silu(x_e @ W_e + b_e) per expert  -> [65536, 2048] f32

Sharding: expert-parallel, 2 experts (= 8192 contiguous sorted tokens) per
core.

Design (weight-stationary): each matmul computes a [128 outs, 512 toks]
PSUM tile: lhsT = w chunk [128, 2(DR), 128 outs], rhs = xT chunk
[128, 2(DR), 512 toks], accumulated over 4 k-pairs.  The output is produced
TRANSPOSED (yt [2048, 8192] f16) and de-transposed on the host.  This makes
the bias per-PARTITION, so the whole PSUM drain is ONE scalar-engine
activation: silu(psum * OUT_SCALE + bias_fp32) reading PSUM directly --
no DVE work at all.  Tokens are processed in stripe-PAIR blocks (1024
tokens) so one [128, 2, 512] two-bank ACT drains a whole ob, keeping the
scalar engine under ~80% busy; y stores ride the sync ring (the scalar
ring is ACT-only) at 4-ob granularity (2KB DRAM runs).

Head shaping: stripe 0 runs first against only out-blocks 0-7 so the
critical preload is half the expert's weights; out-blocks 8-15 of stripe 0
run as a third block against the still-resident x.  Head DMAs are few and
large (2-4KB per-partition lines) because walrus shares completion
semaphores across queues -- many small head DMAs serialize on sem reuse.
6 zero-matmul warmups flip the PE HAM clock-gate while the preload
streams.  The final block's last out-blocks drain per-stripe with small
sync-ring stores to shorten the tail.
"""

import numpy as np
import ml_dtypes

import concourse.bass as bass
import concourse.mybir as mybir
import concourse.tile as tile
from concourse.bass_utils import run_bass_kernel_spmd

N_CORES = 8
N_TOKENS = 65536
D_IN = 1024
D_OUT = 2048
N_EXPERTS = 16
E_PER_CORE = N_EXPERTS // N_CORES        # 2
TOK_PER_CORE = N_TOKENS // N_CORES       # 8192
TOK_PER_EXPERT = N_TOKENS // N_EXPERTS   # 4096

P = 128
KP = 4                     # DoubleRow k-pairs (256 contraction each)
TS = 512                   # token stripe (matmul moving free dim)
N_STRIPES = TOK_PER_CORE // TS           # 16
OBW = 128                  # out-feature block (psum partition dim)
N_OB = D_OUT // OBW        # 16

S_X = 4.0                  # keeps x (std 1) in e4m3 normal range
S_W = 128.0                # keeps W (std ~0.0054) out of e4m3 subnormals
OUT_SCALE = 1.0 / (S_X * S_W)

N_WARMUP_MM = 8

F32 = mybir.dt.float32
F16 = mybir.dt.float16
F8 = mybir.dt.float8e4
NP_F8 = ml_dtypes.float8_e4m3

DR = mybir.MatmulPerfMode.DoubleRow
SILU = mybir.ActivationFunctionType.Silu


def _split_multi_waits(nc):
    """This container's walrus encodes at most ONE sync-wait per instruction;
    hoist extras onto single-wait NoOps inserted just before, same engine."""
    for fn in nc.m.functions:
        for bb in fn.blocks:
            insts = list(bb.instructions)
            out = []
            dirty = False
            for inst in insts:
                si = inst.sync_info
                waits = list(si.on_wait) if si and si.on_wait else []
                if len(waits) > 1:
                    dirty = True
                    for j, w in enumerate(waits[:-1]):
                        nop = mybir.InstNoOp(
                            name=f"{inst.name}-prewait{j}", ins=[], outs=[]
                        )
                        nop.engine = inst.engine
                        nop.sync_info = mybir.SyncInfo(on_wait=[w], on_update=[])
                        out.append(nop)
                    inst.sync_info = mybir.SyncInfo(
                        on_wait=[waits[-1]],
                        on_update=list(si.on_update) if si.on_update else [],
                    )
                out.append(inst)
            if dirty:
                bb.instructions = out


def build_kernel():
    nc = bass.Bass()
    # xt[s, kp, p, j*TS+t] = S_X * X[s*TS+t, kp*256 + j*128 + p]
    xt = nc.dram_tensor("xt", [N_STRIPES, KP, P, 2 * TS], F8,
                        kind="ExternalInput")
    # w[e, ob, p, kp*256 + j*128 + i] = S_W * W_e[kp*256 + j*128 + p, ob*128+i]
    w = nc.dram_tensor("w", [E_PER_CORE, N_OB, P, KP * 2 * OBW], F8,
                       kind="ExternalInput")
    # bb[e, p, ob] = bias[ob*128 + p] (exact fp32, applied inside ACT)
    bb = nc.dram_tensor("bb", [E_PER_CORE, P, N_OB], F32, kind="ExternalInput")
    # transposed output; host does yt.T
    yt = nc.dram_tensor("yt", [D_OUT, TOK_PER_CORE], F16, kind="ExternalOutput")

    # block schedule: (expert, [stripe ids], ob_lo, ob_hi)
    blocks = [
        (0, [0], 0, 8),         # head: small critical preload
        (0, [1], 0, 16),
        (0, [0], 8, 16),        # finish stripe 0 against resident x
        (0, [2, 3], 0, 16),
        (0, [4, 5], 0, 16),
        (0, [6, 7], 0, 16),
        (1, [8, 9], 0, 16),
        (1, [10, 11], 0, 16),
        (1, [12, 13], 0, 16),
        (1, [14, 15], 0, 16),   # tail pair; last obs drain per-stripe
    ]

    with tile.TileContext(nc) as tc:
        with (
            tc.tile_pool(name="persist", bufs=1) as persist,
            tc.tile_pool(name="xs", bufs=5) as xsp,
            tc.tile_pool(name="outs", bufs=4) as outs,
            tc.tile_pool(name="psum", bufs=2, space="PSUM") as psump,
            tc.tile_pool(name="psum1", bufs=3, space="PSUM") as psump1,
        ):
            # --- PE warm-up: matmuls over zeroed scratch, no DMA deps.
            # Sized to keep the PE busy from ~8us until the first real
            # matmul's operands land (~11.5us): an idle PE never reaches
            # the HAM 8/8 clock state and the whole first expert would run
            # at 1.2GHz.
            zs = persist.tile([P, 2, TS], F8, name="warm_src")
            nc.vector.memset(zs[:], 0.0)
            ps_warm = psump1.tile([P, TS], F32, tag="ps1", name="ps_warm")
            for i in range(N_WARMUP_MM):
                nc.tensor.matmul(
                    ps_warm[:],
                    lhsT=zs[:, :, 0:P],
                    rhs=zs[:],
                    start=True, stop=True,
                    perf_mode=DR,
                    skip_group_check=True,
                )

            # --- persistent weight/bias tiles ---
            # e0: obs 0-7 as 2-ob tiles (2KB lines, fine head granularity),
            # obs 8-15 and all of e1 as 4-ob tiles (4KB lines).
            w8d = {q: persist.tile([P, 2, KP, 2, OBW], F8, name=f"w8d_{q}")
                   for q in range(4)}
            w8q = {}
            for e in range(E_PER_CORE):
                q0 = 2 if e == 0 else 0
                for q in range(q0, 4):
                    w8q[(e, q)] = persist.tile([P, 4, KP, 2, OBW], F8,
                                               name=f"w8q_{e}_{q}")
            b_sb = [persist.tile([P, N_OB], F32, name=f"bias_{e}")
                    for e in range(E_PER_CORE)]

            def w_ap(e, ob, kp):
                if e == 0 and ob < 8:
                    return w8d[ob // 2][:, ob % 2, kp, :, :]
                return w8q[(e, ob // 4)][:, ob % 4, kp, :, :]

            def load_w2(q, eng):
                eng.dma_start(
                    w8d[q][:],
                    w[0, 2 * q:2 * q + 2].rearrange(
                        "o p (k j i) -> p o k j i", k=KP, j=2))

            def load_w4(e, q, eng):
                eng.dma_start(
                    w8q[(e, q)][:],
                    w[e, 4 * q:4 * q + 4].rearrange(
                        "o p (k j i) -> p o k j i", k=KP, j=2))

            # x tiles: stripes 0/1 as half-stripe (2 k-pair) tiles for head
            # granularity; the rest as full-stripe tiles (4KB lines).
            xh = {}
            x_tiles = {}

            def load_xhalf(s, h):
                xh[(s, h)] = xsp.tile([P, 2, 2, TS], F8, tag="xh",
                                      name=f"xh_{s}_{h}")
                nc.sync.dma_start(
                    xh[(s, h)][:],
                    xt[s, 2 * h:2 * h + 2].rearrange(
                        "k p (j t) -> p k j t", j=2))

            def load_stripe(s):
                x_tiles[s] = xsp.tile([P, KP, 2, TS], F8, tag="xs",
                                      name=f"xs_{s}")
                nc.sync.dma_start(
                    x_tiles[s][:],
                    xt[s].rearrange("k p (j t) -> p k j t", j=2))

            def x_ap(s, kp):
                if s in (0, 1):
                    return xh[(s, kp // 2)][:, kp % 2, :, :]
                return x_tiles[s][:, kp, :, :]

            # --- head preload, need-ordered, few+large DMAs ---
            # The first-matmul operands land in parallel: x stripe-0 lower
            # half leads the sync queue while w obs0-1 leads the scalar
            # queue.  Biases go on the gpsimd software queue so their
            # completion semaphores don't serialize later hardware-queue
            # loads (walrus rotates a small global semaphore pool).
            load_xhalf(0, 0)             # sync: stripe 0, kp 0-1
            load_w2(0, nc.scalar)        # scalar: obs 0-1
            load_xhalf(0, 1)             # sync: stripe 0, kp 2-3
            load_w2(1, nc.sync)          # obs 2-3
            load_w2(2, nc.scalar)        # obs 4-5
            load_w2(3, nc.scalar)        # obs 6-7
            nc.gpsimd.dma_start(b_sb[0][:], bb[0])
            # sync continues: stripe 1, e0 upper weights
            load_xhalf(1, 0)
            load_xhalf(1, 1)
            load_w4(0, 2, nc.sync)       # obs 8-11
            load_w4(0, 3, nc.sync)       # obs 12-15
            nc.gpsimd.dma_start(b_sb[1][:], bb[1])

            # x/w prefetch emitted on sync at the start of block bi
            prefetch = {
                1: [lambda: load_stripe(2), lambda: load_stripe(3)],
                2: [lambda: load_stripe(4), lambda: load_stripe(5)],
                3: [lambda: load_stripe(6), lambda: load_stripe(7),
                    lambda: load_w4(1, 0, nc.sync),
                    lambda: load_w4(1, 1, nc.sync)],
                4: [lambda: load_stripe(8), lambda: load_stripe(9),
                    lambda: load_w4(1, 2, nc.sync),
                    lambda: load_w4(1, 3, nc.sync)],
                5: [lambda: load_stripe(10), lambda: load_stripe(11)],
                6: [lambda: load_stripe(12), lambda: load_stripe(13)],
                7: [lambda: load_stripe(14), lambda: load_stripe(15)],
            }

            n_blocks = len(blocks)
            for bi, (e, stripes, ob_lo, ob_hi) in enumerate(blocks):
                for fn in prefetch.get(bi, []):
                    fn()
                pair = len(stripes) == 2
                span = len(stripes) * TS
                t0 = stripes[0] * TS
                last_block = bi == n_blocks - 1
                for og in range(ob_lo, ob_hi, 4):
                    obs = list(range(og, min(og + 4, ob_hi)))
                    tail_og = last_block and og + 4 >= ob_hi
                    if not tail_og:
                        tag = "ytp" if pair else "yts"
                        y4 = outs.tile([P, 4, span], F16, tag=tag, name="y4")
                    for oi, ob in enumerate(obs):
                        if pair:
                            ps = psump.tile([P, 2, TS], F32, tag="ps2",
                                            name="ps2")
                            ps_of = [ps[:, 0, :], ps[:, 1, :]]
                        else:
                            ps = psump1.tile([P, TS], F32, tag="ps1",
                                             name="ps1")
                            ps_of = [ps[:]]
                        for kp in range(KP):
                            for si in range(len(stripes)):
                                nc.tensor.matmul(
                                    ps_of[si],
                                    lhsT=w_ap(e, ob, kp),
                                    rhs=x_ap(stripes[si], kp),
                                    start=(kp == 0),
                                    stop=(kp == KP - 1),
                                    perf_mode=DR,
                                )
                        bias_ap = b_sb[e][:, ob:ob + 1]
                        if not tail_og:
                            # one ACT drains the whole ob (both banks)
                            y_dst = y4[:, oi, :]
                            if pair:
                                y_dst = y_dst.rearrange("p (s t) -> p s t",
                                                        s=2)
                            nc.scalar.activation(
                                y_dst, ps[:], SILU, bias=bias_ap,
                                scale=OUT_SCALE)
                        elif ob < ob_hi - 2:
                            # tail obs 12-13: per-ob drain + store
                            y1 = outs.tile([P, 2, TS], F16, tag="ytm",
                                           name="y1")
                            nc.scalar.activation(y1[:], ps[:], SILU,
                                                 bias=bias_ap,
                                                 scale=OUT_SCALE)
                            nc.sync.dma_start(
                                yt[ob * OBW:(ob + 1) * OBW, t0:t0 + span],
                                y1[:])
                        else:
                            # final two obs: per-stripe drains + stores so
                            # the post-last-matmul chain is short
                            for si, s in enumerate(stripes):
                                ys = outs.tile([P, TS], F16, tag="ytt",
                                               name="ys")
                                nc.scalar.activation(ys[:], ps_of[si], SILU,
                                                     bias=bias_ap,
                                                     scale=OUT_SCALE)
                                nc.sync.dma_start(
                                    yt[ob * OBW:(ob + 1) * OBW,
                                       s * TS:(s + 1) * TS],
                                    ys[:])
                    if not tail_og:
                        dst = yt[og * OBW:(og + 4) * OBW,
                                 t0:t0 + span].rearrange(
                                     "(o p) t -> p o t", p=P)
                        nc.sync.dma_start(dst, y4[:])

    _split_multi_waits(nc)
    return nc


_NC_CACHE = None


def _get_nc():
    global _NC_CACHE
    if _NC_CACHE is None:
        _NC_CACHE = build_kernel()
    return _NC_CACHE


def _in_maps(sorted_features, routing_matrix, routing_bias):
    maps = []
    for c in range(N_CORES):
        rows = slice(c * TOK_PER_CORE, (c + 1) * TOK_PER_CORE)
        es = slice(c * E_PER_CORE, (c + 1) * E_PER_CORE)
        # [s, kp, p, j*TS+t] = S_X * X[s*TS+t, kp*256 + j*128 + p]
        xt_c = np.ascontiguousarray(
            (sorted_features[rows] * S_X)
            .astype(NP_F8)
            .reshape(N_STRIPES, TS, KP, 2, P)   # [s, t, kp, j, p]
            .transpose(0, 2, 4, 3, 1)           # [s, kp, p, j, t]
            .reshape(N_STRIPES, KP, P, 2 * TS)
        )
        # [e, ob, p, kp*256 + j*128 + i]
        w_c = np.ascontiguousarray(
            (routing_matrix[:, :, es] * S_W)
            .astype(NP_F8)
            .transpose(2, 0, 1)                      # [e, d_in, d_out]
            .reshape(E_PER_CORE, KP, 2, P, N_OB, OBW)  # [e,kp,j,p,ob,i]
            .transpose(0, 4, 3, 1, 2, 5)             # [e, ob, p, kp, j, i]
            .reshape(E_PER_CORE, N_OB, P, KP * 2 * OBW)
        )
        # [e, p, ob] = bias[ob*128 + p]  (exact fp32)
        b_c = np.ascontiguousarray(
            routing_bias[:, es].T                    # [e, d_out]
            .reshape(E_PER_CORE, N_OB, P)
            .transpose(0, 2, 1)
            .astype(np.float32)
        )
        maps.append({"xt": xt_c, "w": w_c, "bb": b_c})
    return maps


def run(sorted_features, routing_matrix, routing_bias, **run_kwargs):
    nc = _get_nc()
    maps = _in_maps(sorted_features, routing_matrix, routing_bias)
    res = run_bass_kernel_spmd(nc, maps, core_ids=list(range(N_CORES)),
                               **run_kwargs)
    out = np.empty((N_TOKENS, D_OUT), dtype=np.float32)
    for c in range(N_CORES):
        yt_c = np.asarray(res.results[c]["yt"])
        out[c * TOK_PER_CORE:(c + 1) * TOK_PER_CORE] = yt_c.T
    return out, res


def kernel(sorted_features, expert_ids_sorted, routing_matrix, routing_bias):
    assert sorted_features.shape == (N_TOKENS, D_IN)
    assert routing_matrix.shape == (D_IN, D_OUT, N_EXPERTS)
    assert routing_bias.shape == (D_OUT, N_EXPERTS)
    out, _ = run(
        np.asarray(sorted_features, dtype=np.float32),
        np.asarray(routing_matrix, dtype=np.float32),
        np.asarray(routing_bias, dtype=np.float32),
    )
    return out
